# revision 1
# baseline (speedup 1.0000x reference)
"""Trainium2 Bass kernel for GQA attention with RoPE (B=2, S=1024, HID=2048,
16 q heads / 4 kv heads, head dim 128, causal).

Sharding: 8 cores = 2 batches x 4 kv-head groups. Core c = b*4 + g handles
batch b and kv head g (query heads 4g..4g+3). Each core computes a partial
output y_part = attn_heads @ wo_shard; the host sums the 4 partials per batch.

Per-core dataflow (matmuls fp32r, moving free dim >= 256):
  Phase A (per 128-row chunk g, software-pipelined 2 deep):
    x chunk --PE transpose--> xT --mm--> q, [k|v] (natural); RoPE on DVE;
    PE transpose q_rope/k_rope -> persistent qT[d,h,s], kT[d,s], v[s,d].
  Phase B/C (per 256-col macro tile, heads pipelined one deep):
    scoresT[sk,sq] = kT_chunk.T @ qT ; expS = exp(scale*s + mask)  (ACT)
    denom_rep = ones.T @ expS ; U^T = v.T-free @ expS   (PE, accumulated)
    rec = exp(-ln(denom))  (ACT) ; uT = U^T * rec  (DVE, fused with copy)
    y = sum_h uT_h.T @ wo_h  (PE) -> SBUF -> DRAM
"""

import sys

import numpy as np

for _p in ("/opt/trn_rl_repo", "/root/.axon_site/_ro/trn_rl_repo"):
    if _p not in sys.path:
        sys.path.append(_p)

from contextlib import ExitStack

import concourse.bass as bass
import concourse.mybir as mybir
from concourse import bacc
from concourse.masks import make_identity
from concourse.tile import TileContext

P = 128           # partitions / head dim / seq chunk
S = 1024          # sequence length
HID = 2048        # model dim
NH = 4            # query heads per core
D = 128           # head dim
TQ = 256          # query macro-tile (matmul moving free dim)
NT = S // TQ      # 4 macro tiles
KC = HID // P     # 16 contraction chunks
NSK = S // P      # 8 key chunks
NG = S // P       # 8 row chunks
F32 = mybir.dt.float32
F32R = mybir.dt.float32r
SCALE = 1.0 / float(np.sqrt(D))
NEG = -30000.0
AL = mybir.AluOpType
AF = mybir.ActivationFunctionType

N_CORES = 8
B = 2
N_KV = 4


def build_nc():
    nc = bacc.Bacc("TRN2", target_bir_lowering=False, debug=False)
    x_d = nc.declare_dram_parameter("x", [S, HID], F32R, isOutput=False)
    cos_d = nc.declare_dram_parameter("cos", [S, D], F32, isOutput=False)
    sin_d = nc.declare_dram_parameter("sin", [S, D], F32, isOutput=False)
    wq_d = nc.declare_dram_parameter("wq", [HID, NH * D], F32R, isOutput=False)
    wk_d = nc.declare_dram_parameter("wk", [HID, D], F32R, isOutput=False)
    wv_d = nc.declare_dram_parameter("wv", [HID, D], F32R, isOutput=False)
    wo_d = nc.declare_dram_parameter("wo", [NH * D, HID], F32R, isOutput=False)
    out_d = nc.declare_dram_parameter("out", [S, HID], F32, isOutput=True)

    with TileContext(nc) as tc, ExitStack() as ctx:
        consts = ctx.enter_context(tc.tile_pool(name="consts", bufs=1))
        wpool = ctx.enter_context(tc.tile_pool(name="wpool", bufs=1))
        persist = ctx.enter_context(tc.tile_pool(name="persist", bufs=1))

        # ---- constants ----
        ident_f32 = consts.tile([P, P], F32, tag="ident_f32")
        make_identity(nc, ident_f32)
        ident = consts.tile([P, P], F32R, tag="ident")
        nc.vector.tensor_copy(ident, ident_f32)
        ones_f32 = consts.tile([P, P], F32, tag="ones_f32")
        nc.vector.memset(ones_f32, 1.0)
        ones = consts.tile([P, P], F32R, tag="ones")
        nc.vector.tensor_copy(ones, ones_f32)

        # ---- weights (partition-chunked layouts), interleaved with x loads ----
        wq_sb = wpool.tile([P, KC, NH * D], F32R, tag="wq")
        wq_r = wq_d[:].rearrange("(c p) n -> p c n", p=P)
        wkv_sb = wpool.tile([P, KC, 2 * D], F32R, tag="wkv")
        wo_sb = wpool.tile([P, NH, HID], F32R, tag="wo")
        wo_r = wo_d[:].rearrange("(h p) n -> p h n", p=P)
        cos_sb = wpool.tile([P, NG, D], F32, tag="cos")
        sin_sb = wpool.tile([P, NG, D], F32, tag="sin")

        # persistent transposed activations
        qT_all = persist.tile([P, NH, S], F32R, tag="qT")   # [d, h, sq]
        kT = persist.tile([P, S], F32R, tag="kT")           # [d, sk]
        vv = persist.tile([P, NSK, D], F32R, tag="vv")      # v natural [sk, d]

        H2 = D // 2

        def rope(dst, src, g, tmp_tag, wk):
            """dst = src*cos + rotate_half(src)*sin, natural layout [P, D]."""
            cos_g = cos_sb[:, g, :]
            sin_g = sin_sb[:, g, :]
            tmp = wk.tile([P, D], F32, tag=tmp_tag)
            nc.vector.scalar_tensor_tensor(
                out=tmp[:, 0:H2], in0=src[:, H2:D], scalar=-1.0,
                in1=sin_g[:, 0:H2], op0=AL.mult, op1=AL.mult,
            )
            nc.vector.tensor_tensor(
                out=tmp[:, H2:D], in0=src[:, 0:H2], in1=sin_g[:, H2:D], op=AL.mult
            )
            nc.vector.tensor_tensor(out=dst, in0=src, in1=cos_g, op=AL.mult)
            nc.vector.tensor_tensor(out=dst, in0=dst, in1=tmp, op=AL.add)

        # ================= fused pipeline =================
        pa = ctx.enter_context(tc.tile_pool(name="pa", bufs=2))
        pb = ctx.enter_context(tc.tile_pool(name="pb", bufs=2))
        ps_mega = ctx.enter_context(tc.tile_pool(name="ps_mega", bufs=6, space="PSUM"))
        ps_qkv = ctx.enter_context(tc.tile_pool(name="ps_qkv", bufs=1, space="PSUM"))

        # dummy matmuls to lift the PE HAM clock gate to 8/8 while the
        # first x/weight DMAs are still in flight
        warm_ps = ps_mega.tile([P, 512], F32, tag="mega", name="warm")
        for _ in range(40):
            nc.tensor.matmul(warm_ps[:, 0:P], ones, ones, start=True, stop=True)
        warm_drain = pa.tile([P, 4], F32, tag="warmdrain", bufs=1)
        nc.vector.tensor_copy(warm_drain, warm_ps[:, 0:4])

        # causal masks for the two diagonal-straddling chunk positions
        m12 = consts.tile([P, 2 * TQ], F32, tag="m12")
        nc.gpsimd.memset(m12, 0.0)
        nc.gpsimd.affine_select(
            out=m12[:, 0:TQ], in_=m12[:, 0:TQ], compare_op=AL.is_ge, fill=NEG,
            base=0, pattern=[[1, TQ]], channel_multiplier=-1,
        )
        nc.gpsimd.affine_select(
            out=m12[:, TQ : 2 * TQ], in_=m12[:, TQ : 2 * TQ],
            compare_op=AL.is_ge, fill=NEG,
            base=-P, pattern=[[1, TQ]], channel_multiplier=-1,
        )

        x_tiles = [None] * NG
        pend = [None] * NG  # g -> (q_ps3, kv_ps, xT)

        def emit_xdma(g):
            x_nat = pa.tile([P, HID], F32R, tag="xnat", bufs=3)
            nc.sync.dma_start(out=x_nat, in_=x_d[g * P : (g + 1) * P, :])
            x_tiles[g] = x_nat

        # DMA order: x0, wq(2), wkv, x1, cos, sin, x2.., wo(4) trailing
        emit_xdma(0)
        nc.sync.dma_start(out=wq_sb[:, 0:4, :], in_=wq_r[:, 0:4, :])
        nc.sync.dma_start(out=wq_sb[:, 4:8, :], in_=wq_r[:, 4:8, :])
        emit_xdma(1)
        nc.sync.dma_start(out=wq_sb[:, 8:12, :], in_=wq_r[:, 8:12, :])
        nc.sync.dma_start(out=wq_sb[:, 12:16, :], in_=wq_r[:, 12:16, :])
        nc.sync.dma_start(
            out=wkv_sb[:, :, 0:D], in_=wk_d[:].rearrange("(c p) n -> p c n", p=P)
        )
        nc.sync.dma_start(
            out=wkv_sb[:, :, D : 2 * D],
            in_=wv_d[:].rearrange("(c p) n -> p c n", p=P),
        )
        emit_xdma(2)
        nc.sync.dma_start(
            out=cos_sb, in_=cos_d[:].rearrange("(c p) d -> p c d", p=P)
        )
        nc.sync.dma_start(
            out=sin_sb, in_=sin_d[:].rearrange("(c p) d -> p c d", p=P)
        )
        wo_next = [0]

        def emit_wo_dma():
            h = wo_next[0]
            if h < NH:
                nc.sync.dma_start(out=wo_sb[:, h, :], in_=wo_r[:, h, :])
                wo_next[0] += 1

        def transposes(g):
            """x chunk -> xT (PE transpose + DVE cast-copy)."""
            x_nat = x_tiles[g]
            xT = pa.tile([P, KC, P], F32R, tag="xT", bufs=2)
            xT_flat = xT.rearrange("p c d -> p (c d)")
            for kb in range(KC // 4):
                tp_ps = ps_mega.tile([P, 4 * P], F32R, tag="mega", name="tp")
                for j in range(4):
                    k = 4 * kb + j
                    nc.tensor.transpose(
                        tp_ps[:, j * P : (j + 1) * P],
                        x_nat[:, k * P : (k + 1) * P],
                        ident,
                    )
                if kb % 2 == 0:
                    nc.vector.tensor_copy(
                        xT_flat[:, kb * 4 * P : (kb + 1) * 4 * P], tp_ps
                    )
                else:
                    nc.scalar.activation(
                        out=xT_flat[:, kb * 4 * P : (kb + 1) * 4 * P], in_=tp_ps,
                        func=AF.Copy,
                    )
            return xT

        def proj(g, xT):
            """q and kv projections for chunk g (PE, accumulating);
            result copied straight out to SBUF to free the PSUM bank."""
            qkv_ps = ps_qkv.tile([P, NH * D + 2 * D], F32, tag="qkv")
            q_ps = qkv_ps[:, 0 : NH * D]
            kv_ps = qkv_ps[:, NH * D : NH * D + 2 * D]
            for k in range(KC):
                nc.tensor.matmul(
                    q_ps, xT[:, k, :], wq_sb[:, k, :],
                    start=(k == 0), stop=(k == KC - 1),
                )
            for k in range(KC):
                nc.tensor.matmul(
                    kv_ps, xT[:, k, :], wkv_sb[:, k, :],
                    start=(k == 0), stop=(k == KC - 1),
                )
            qkv_sb = pa.tile([P, NH * D + 2 * D], F32, tag="qkvsb")
            nc.scalar.activation(out=qkv_sb, in_=qkv_ps, func=AF.Copy)
            return qkv_sb

        def rope_stage(g, qkv_sb):
            """RoPE on q heads + k (DVE), v copy-out."""
            q3 = qkv_sb[:, 0 : NH * D].rearrange("p (h d) -> p h d", h=NH)
            kv_ps = qkv_sb[:, NH * D : NH * D + 2 * D]
            q_rope = pa.tile([P, NH, D], F32R, tag="qrope")
            for h in range(NH):
                rope(q_rope[:, h, :], q3[:, h, :], g, "tmq", pa)
            k_rope = pa.tile([P, D], F32R, tag="krope")
            rope(k_rope, kv_ps[:, 0:D], g, "tmk", pa)
            nc.vector.tensor_copy(vv[:, g, :], kv_ps[:, D : 2 * D])
            return q_rope, k_rope

        def rope_transpose(g, q_rope, k_rope):
            """Transpose RoPE'd q/k into persistent qT_all / kT."""
            tq_ps = ps_mega.tile([P, 4 * P], F32R, tag="mega", name="tq")
            for h in range(NH):
                nc.tensor.transpose(
                    tq_ps[:, h * P : (h + 1) * P], q_rope[:, h, :], ident
                )
            nc.vector.tensor_copy(
                qT_all[:, :, g * P : (g + 1) * P],
                tq_ps.rearrange("p (h d) -> p h d", h=NH),
            )
            tk_ps = ps_mega.tile([P, 4 * P], F32R, tag="mega", name="tk")
            nc.tensor.transpose(tk_ps[:, 0:P], k_rope, ident)
            nc.vector.tensor_copy(kT[:, g * P : (g + 1) * P], tk_ps[:, 0:P])

        # 2-deep software pipeline over chunks
        ropes = [None] * NG
        attn_todo = []  # deferred attention head-steps, emitted between A work

        def emit_phase_a(g):
            if g >= 2:
                gg = g - 2
                sc = nc.named_scope(f"rope_{gg}"); sc.__enter__()
                ropes[gg] = rope_stage(gg, pend[gg][1])
                sc.__exit__(None, None, None)
            if g < NG:
                if g + 3 < NG:
                    emit_xdma(g + 3)
                if g >= 3:
                    emit_wo_dma()
                    emit_wo_dma()
                sc = nc.named_scope(f"tp_{g}"); sc.__enter__()
                xT = transposes(g)
                sc.__exit__(None, None, None)
                pend[g] = [xT, None, None]
            if g >= 1 and g - 1 < NG:
                gg = g - 1
                sc = nc.named_scope(f"proj_{gg}"); sc.__enter__()
                qkv_sb = proj(gg, pend[gg][0])
                sc.__exit__(None, None, None)
                pend[gg][1] = qkv_sb
            if g >= 2:
                gg = g - 2
                sc = nc.named_scope(f"ropeT_{gg}"); sc.__enter__()
                rope_transpose(gg, *ropes[gg])
                sc.__exit__(None, None, None)
                pend[gg] = None


        def scores_head(t, h):
            """scoresT + exp for head h of macro tile t -> expst tile.

            Chunk pairs share one full PSUM bank so the causal mask is a
            single DVE add and exp is one ACT op per pair."""
            qT_h = qT_all[:, h, t * TQ : (t + 1) * TQ]
            expst = pb.tile([P, NSK, TQ], F32R, tag="expst", bufs=3)
            expst_flat = expst.rearrange("p c f -> p (c f)")
            for pi in range(t + 1):
                s_ps = ps_mega.tile([P, 2 * TQ], F32, tag="mega", name="s")
                for half in range(2):
                    ik = 2 * pi + half
                    nc.tensor.matmul(
                        s_ps[:, half * TQ : (half + 1) * TQ],
                        kT[:, ik * P : (ik + 1) * P], qT_h,
                        start=True, stop=True,
                    )
                if pi == t:
                    nc.vector.tensor_tensor(out=s_ps, in0=s_ps, in1=m12, op=AL.add)
                nc.scalar.activation(
                    out=expst_flat[:, pi * 2 * TQ : (pi + 1) * 2 * TQ],
                    in_=s_ps, func=AF.Exp, scale=SCALE,
                )
            return expst

        def dnpv_head(t, h, expst, uT_t):
            """denominator + PV matmuls, then normalize into uT_t (DVE)."""
            nsk = 2 * (t + 1)
            u_ps = ps_mega.tile([P, 2 * TQ], F32, tag="mega", name="u")[:, 0:TQ]
            den_ps = ps_mega.tile([P, 2 * TQ], F32, tag="mega", name="den")[:, 0:TQ]
            for ik in range(nsk):
                nc.tensor.matmul(
                    den_ps, ones, expst[:, ik, :],
                    start=(ik == 0), stop=(ik == nsk - 1),
                )
            rec = pb.tile([P, TQ], F32, tag="rec", bufs=2)
            nc.vector.reciprocal(rec, den_ps)
            for ik in range(nsk):
                nc.tensor.matmul(
                    u_ps, vv[:, ik, :], expst[:, ik, :],
                    start=(ik == 0), stop=(ik == nsk - 1),
                )
            nc.vector.tensor_tensor(
                out=uT_t[:, h, :], in0=u_ps, in1=rec, op=AL.mult
            )

        def wo_stage(t, uT_t):
            for sub in range(2):
                g = 2 * t + sub
                for n in range(HID // 512):
                    y_ps = ps_mega.tile([P, 512], F32, tag="mega", name="y")
                    for h in range(NH):
                        nc.tensor.matmul(
                            y_ps,
                            uT_t[:, h, sub * P : (sub + 1) * P],
                            wo_sb[:, h, n * 512 : (n + 1) * 512],
                            start=(h == 0), stop=(h == NH - 1),
                        )
                    y_sb = pb.tile([P, 512], F32, tag="ysb", bufs=2)
                    nc.vector.tensor_copy(y_sb, y_ps)
                    nc.gpsimd.dma_start(
                        out=out_d[g * P : (g + 1) * P, n * 512 : (n + 1) * 512],
                        in_=y_sb,
                    )

        # attention head-steps, pipelined one deep with wo lagging two
        # steps; consumed interleaved with phase-A iterations
        steps = [(t, h) for t in range(NT) for h in range(NH)]
        uts = {}
        att_i = [0]

        def emit_attention_step():
            i = att_i[0]
            if i >= len(steps) + 2:
                return False
            if i < len(steps):
                t, h = steps[i]
                if h == 0:
                    uts[t] = pb.tile([P, NH, TQ], F32R, tag="uT", name=f"uT{t}")
                sc = nc.named_scope(f"sc_{t}_{h}"); sc.__enter__()
                uts[(t, h)] = scores_head(t, h)
                sc.__exit__(None, None, None)
            if 1 <= i < len(steps) + 1:
                t, h = steps[i - 1]
                sc = nc.named_scope(f"dnpv_{t}_{h}"); sc.__enter__()
                dnpv_head(t, h, uts.pop((t, h)), uts[t])
                sc.__exit__(None, None, None)
            if i >= 2 and (i - 2) % NH == NH - 1:
                t = steps[i - 2][0]
                sc = nc.named_scope(f"wo_{t}"); sc.__enter__()
                wo_stage(t, uts.pop(t))
                sc.__exit__(None, None, None)
            att_i[0] += 1
            return True

        # drive: phase-A iteration g, then any attention steps whose
        # inputs (kT/v/qT up to chunk 2t+1) are complete after ropeT_{g-2}
        for g in range(NG + 2):
            emit_phase_a(g)
            done_g = g - 2  # ropeT for this chunk just emitted
            while att_i[0] < len(steps) + 2:
                i = att_i[0]
                if i < len(steps):
                    t, _h = steps[i]
                    if 2 * t + 1 > done_g:
                        break
                emit_attention_step()
        emit_wo_dma()
        emit_wo_dma()
        emit_wo_dma()
        emit_wo_dma()
        while emit_attention_step():
            pass

    nc.compile()
    return nc


def shard_inputs(x, cos, sin, wq, wk, wv, wo):
    """Build per-core input maps: core = b*4 + g."""
    in_maps = []
    for c in range(N_CORES):
        b, g = divmod(c, N_KV)
        in_maps.append(
            {
                "x": np.ascontiguousarray(x[b]),
                "cos": np.ascontiguousarray(cos),
                "sin": np.ascontiguousarray(sin),
                "wq": np.ascontiguousarray(wq[:, g * NH * D : (g + 1) * NH * D]),
                "wk": np.ascontiguousarray(wk[:, g * D : (g + 1) * D]),
                "wv": np.ascontiguousarray(wv[:, g * D : (g + 1) * D]),
                "wo": np.ascontiguousarray(wo[g * NH * D : (g + 1) * NH * D, :]),
            }
        )
    return in_maps


_NC_CACHE = {}


def get_nc():
    if "nc" not in _NC_CACHE:
        _NC_CACHE["nc"] = build_nc()
    return _NC_CACHE["nc"]


def kernel(x, cos, sin, wq, wk, wv, wo, _trace=False):
    from concourse.bass_utils import run_bass_kernel_spmd

    x = np.asarray(x, dtype=np.float32)
    cos = np.asarray(cos, dtype=np.float32)
    sin = np.asarray(sin, dtype=np.float32)
    wq = np.asarray(wq, dtype=np.float32)
    wk = np.asarray(wk, dtype=np.float32)
    wv = np.asarray(wv, dtype=np.float32)
    wo = np.asarray(wo, dtype=np.float32)

    nc = get_nc()
    in_maps = shard_inputs(x, cos, sin, wq, wk, wv, wo)
    res = run_bass_kernel_spmd(nc, in_maps, list(range(N_CORES)), trace=_trace)
    parts = [np.asarray(res.results[c]["out"], dtype=np.float32) for c in range(N_CORES)]
    y = np.stack(
        [sum(parts[b * N_KV + g] for g in range(N_KV)) for b in range(B)], axis=0
    )
    if _trace:
        kernel.last_result = res
    return y



# revision 4
# speedup vs baseline: 1.1122x; 1.1122x over previous
"""Trainium2 Bass kernel for GQA attention with RoPE (B=2, S=1024, HID=2048,
16 q heads / 4 kv heads, head dim 128, causal).

Sharding: 8 cores = 2 batches x 4 kv-head groups. Core c = b*4 + g handles
batch b and kv head g (query heads 4g..4g+3). Each core computes a partial
output y_part = attn_heads @ wo_shard; the host sums the 4 partials per batch.

All matmul operands are bf16 (weights cast on-chip from the fp32 DMA) so the
PE gets FWL (fast weight load) on every stationary operand and bf16 1c/row
streaming; PSUM accumulation stays fp32.  Key layout choices:

  Phase A (per 128-row chunk g, software-pipelined 2 deep):
    x chunk (fp32) --DVE cast--> x16 --PE transpose (bf16)--> xT
    qkv = xT.T @ [wq|wk|wv] (one 768-wide moving operand); RoPE on DVE in
    bf16; PE transpose q_rope/k_rope -> persistent qT[d,h,s], kT[d,s], v[s,d].
  Phase B (per (macro tile t, head pair hp), pipelined one step deep):
    per 128-key chunk ik: scoresT[sk,(2h,sq)] = kT_chunk.T @ qT_hp (512F)
    expS = exp(scale*s) (ACT) -> bf16; causal masking via gpsimd
    affine_select zero-fill on the two diagonal chunks (replaces additive
    -inf mask); denom = ones.T @ expS and U^T = v.T @ expS accumulate on PE;
    rec = exp(-ln(denom)) on ACT (DVE reciprocal is an 8-pass iterative op);
    uT = U^T * rec (DVE, bf16 out).
  Phase C: y = sum_h uT_h.T @ wo_h (PE) -> SBUF bf16 -> DRAM bf16 (host
    upcasts and sums partials in fp32).
"""

import sys

import numpy as np

for _p in ("/opt/trn_rl_repo", "/root/.axon_site/_ro/trn_rl_repo"):
    if _p not in sys.path:
        sys.path.append(_p)

from contextlib import ExitStack

import concourse.bass as bass
import concourse.mybir as mybir
from concourse import bacc
from concourse.masks import make_identity
from concourse.tile import TileContext

P = 128           # partitions / head dim / seq chunk
S = 1024          # sequence length
HID = 2048        # model dim
NH = 4            # query heads per core
D = 128           # head dim
TQ = 256          # query macro-tile
NT = S // TQ      # 4 macro tiles
KC = HID // P     # 16 contraction chunks
NSK = S // P      # 8 key chunks
NG = S // P       # 8 row chunks
NHP = NH // 2     # head pairs per core
F32 = mybir.dt.float32
BF16 = mybir.dt.bfloat16
SCALE = 1.0 / float(np.sqrt(D))
AL = mybir.AluOpType
AF = mybir.ActivationFunctionType

N_CORES = 8
B = 2
N_KV = 4

QKV = NH * D + 2 * D   # 768: q(512) | k(128) | v(128)


def build_nc():
    nc = bacc.Bacc("TRN2", target_bir_lowering=False, debug=False)
    x_d = nc.declare_dram_parameter("x", [S, HID], F32, isOutput=False)
    cos_d = nc.declare_dram_parameter("cos", [S, D], F32, isOutput=False)
    sin_d = nc.declare_dram_parameter("sin", [S, D], F32, isOutput=False)
    wq_d = nc.declare_dram_parameter("wq", [HID, NH * D], F32, isOutput=False)
    wk_d = nc.declare_dram_parameter("wk", [HID, D], F32, isOutput=False)
    wv_d = nc.declare_dram_parameter("wv", [HID, D], F32, isOutput=False)
    wo_d = nc.declare_dram_parameter("wo", [NH * D, HID], F32, isOutput=False)
    out_d = nc.declare_dram_parameter("out", [S, HID], BF16, isOutput=True)

    with TileContext(nc) as tc, ExitStack() as ctx:
        consts = ctx.enter_context(tc.tile_pool(name="consts", bufs=1))
        wpool = ctx.enter_context(tc.tile_pool(name="wpool", bufs=1))
        stage = ctx.enter_context(tc.tile_pool(name="stage", bufs=2))
        persist = ctx.enter_context(tc.tile_pool(name="persist", bufs=1))

        # ---- constants ----
        ident = consts.tile([P, P], BF16, tag="ident")
        make_identity(nc, ident)
        ones = consts.tile([P, P], BF16, tag="ones")
        nc.vector.memset(ones, 1.0)

        # ---- weights: fp32 staging DMA -> bf16 SBUF cast (DVE) ----
        wqkv_sb = wpool.tile([P, KC, QKV], BF16, tag="wqkv")
        wo_sb = wpool.tile([P, NH, HID], BF16, tag="wo")
        cos_sb = wpool.tile([P, NG, D], BF16, tag="cos")
        sin_sb = wpool.tile([P, NG, D], BF16, tag="sin")

        wq_r = wq_d[:].rearrange("(c p) n -> p c n", p=P)
        wk_r = wk_d[:].rearrange("(c p) n -> p c n", p=P)
        wv_r = wv_d[:].rearrange("(c p) n -> p c n", p=P)
        wo_r = wo_d[:].rearrange("(h p) n -> p h n", p=P)

        def load_wq_round(r):
            st = stage.tile([P, 4, NH * D], F32, tag="wstage", name=f"wqs{r}")
            nc.sync.dma_start(out=st, in_=wq_r[:, 4 * r : 4 * (r + 1), :])
            nc.vector.tensor_copy(
                wqkv_sb[:, 4 * r : 4 * (r + 1), 0 : NH * D], st
            )

        def load_wkv(which):
            src, off = (wk_r, NH * D) if which == "k" else (wv_r, NH * D + D)
            st = stage.tile([P, KC, D], F32, tag="wstage", name=f"w{which}s")
            nc.sync.dma_start(out=st, in_=src)
            nc.vector.tensor_copy(wqkv_sb[:, :, off : off + D], st)

        def load_cs(which):
            src, dst = (cos_d, cos_sb) if which == "c" else (sin_d, sin_sb)
            st = stage.tile([P, NG, D], F32, tag="csstage", name=f"{which}s")
            nc.sync.dma_start(out=st, in_=src[:].rearrange("(c p) d -> p c d", p=P))
            nc.vector.tensor_copy(dst, st)

        wo_next = [0]

        def load_wo():
            h = wo_next[0]
            if h < NH:
                st = stage.tile([P, HID], F32, tag="wostage", name=f"wos{h}")
                nc.sync.dma_start(out=st, in_=wo_r[:, h, :])
                nc.vector.tensor_copy(wo_sb[:, h, :], st)
                wo_next[0] += 1

        # persistent transposed activations (all bf16)
        qT_all = persist.tile([P, NH, S], BF16, tag="qT")   # [d, h, sq]
        kT = persist.tile([P, S], BF16, tag="kT")           # [d, sk]
        vv = persist.tile([P, NSK, D], BF16, tag="vv")      # v natural [sk, d]

        H2 = D // 2

        def rope(dst, src, g, tmp_tag, wk):
            """dst = src*cos + rotate_half(src)*sin, natural layout [P, D]."""
            cos_g = cos_sb[:, g, :]
            sin_g = sin_sb[:, g, :]
            tmp = wk.tile([P, D], BF16, tag=tmp_tag)
            nc.vector.scalar_tensor_tensor(
                out=tmp[:, 0:H2], in0=src[:, H2:D], scalar=-1.0,
                in1=sin_g[:, 0:H2], op0=AL.mult, op1=AL.mult,
            )
            nc.vector.tensor_tensor(
                out=tmp[:, H2:D], in0=src[:, 0:H2], in1=sin_g[:, H2:D], op=AL.mult
            )
            nc.vector.tensor_tensor(out=dst, in0=src, in1=cos_g, op=AL.mult)
            nc.vector.tensor_tensor(out=dst, in0=dst, in1=tmp, op=AL.add)

        # ================= fused pipeline =================
        pa = ctx.enter_context(tc.tile_pool(name="pa", bufs=2))
        pb = ctx.enter_context(tc.tile_pool(name="pb", bufs=2))
        ps_mega = ctx.enter_context(tc.tile_pool(name="ps_mega", bufs=6, space="PSUM"))
        ps_qkv = ctx.enter_context(tc.tile_pool(name="ps_qkv", bufs=1, space="PSUM"))

        # dummy matmuls to lift the PE HAM clock gate to 8/8 while the
        # first x/weight DMAs are still in flight
        warm_ps = ps_mega.tile([P, 512], F32, tag="mega", name="warm")
        for _ in range(40):
            nc.tensor.matmul(warm_ps[:, 0:P], ones, ones, start=True, stop=True)
        warm_drain = pa.tile([P, 4], F32, tag="warmdrain", bufs=1)
        nc.vector.tensor_copy(warm_drain, warm_ps[:, 0:4])

        x_tiles = [None] * NG
        pend = [None] * NG  # g -> [xT, qkv_sb]

        def emit_xdma(g):
            x_nat = pa.tile([P, HID], F32, tag="xnat", bufs=3)
            nc.sync.dma_start(out=x_nat, in_=x_d[g * P : (g + 1) * P, :])
            x_tiles[g] = x_nat

        # DMA order: x0, wq rounds + casts, x1, wkv, x2, cos/sin, x3.., wo
        emit_xdma(0)
        load_wq_round(0)
        load_wq_round(1)
        emit_xdma(1)
        load_wq_round(2)
        load_wq_round(3)
        load_wkv("k")
        load_wkv("v")
        emit_xdma(2)
        load_cs("c")
        load_cs("s")

        def transposes(g):
            """x chunk -> cast bf16 -> xT via PE transposes."""
            x_nat = x_tiles[g]
            x16 = pa.tile([P, HID], BF16, tag="x16", bufs=2)
            nc.vector.tensor_copy(x16, x_nat)
            xT = pa.tile([P, KC, P], BF16, tag="xT", bufs=2)
            xT_flat = xT.rearrange("p c d -> p (c d)")
            for kb in range(2):
                tp_ps = ps_mega.tile([P, 8, P], BF16, tag="mega", name="tp")
                for j in range(8):
                    k = 8 * kb + j
                    nc.tensor.transpose(
                        tp_ps[:, j, :], x16[:, k * P : (k + 1) * P], ident
                    )
                tp_flat = tp_ps.rearrange("p c d -> p (c d)")
                if kb == 0:
                    nc.vector.tensor_copy(
                        xT_flat[:, 0 : 8 * P], tp_flat
                    )
                else:
                    nc.scalar.activation(
                        out=xT_flat[:, 8 * P : 16 * P], in_=tp_flat, func=AF.Copy
                    )
            return xT

        def proj(g, xT):
            """qkv projection: q (512F) and kv (256F) accumulation groups
            (a single matmul's PSUM output must fit one 512-fp32 bank)."""
            qkv_ps = ps_qkv.tile([P, QKV], F32, tag="qkv")
            q_ps = qkv_ps[:, 0 : NH * D]
            kv_ps = qkv_ps[:, NH * D : QKV]
            for k in range(KC):
                nc.tensor.matmul(
                    q_ps, xT[:, k, :], wqkv_sb[:, k, 0 : NH * D],
                    start=(k == 0), stop=(k == KC - 1),
                )
            for k in range(KC):
                nc.tensor.matmul(
                    kv_ps, xT[:, k, :], wqkv_sb[:, k, NH * D : QKV],
                    start=(k == 0), stop=(k == KC - 1),
                )
            qkv_sb = pa.tile([P, QKV], BF16, tag="qkvsb")
            nc.scalar.activation(out=qkv_sb, in_=qkv_ps, func=AF.Copy)
            return qkv_sb

        def rope_stage(g, qkv_sb):
            """RoPE on q heads + k (DVE, bf16), v copy-out."""
            q3 = qkv_sb[:, 0 : NH * D].rearrange("p (h d) -> p h d", h=NH)
            q_rope = pa.tile([P, NH, D], BF16, tag="qrope")
            for h in range(NH):
                rope(q_rope[:, h, :], q3[:, h, :], g, "tmq", pa)
            k_rope = pa.tile([P, D], BF16, tag="krope")
            rope(k_rope, qkv_sb[:, NH * D : NH * D + D], g, "tmk", pa)
            nc.vector.tensor_copy(
                vv[:, g, :], qkv_sb[:, NH * D + D : NH * D + 2 * D]
            )
            return q_rope, k_rope

        def rope_transpose(g, q_rope, k_rope):
            """Transpose RoPE'd q/k into persistent qT_all / kT."""
            t_ps = ps_mega.tile([P, 8, P], BF16, tag="mega", name="tq")
            for h in range(NH):
                nc.tensor.transpose(t_ps[:, h, :], q_rope[:, h, :], ident)
            nc.tensor.transpose(t_ps[:, NH, :], k_rope, ident)
            nc.vector.tensor_copy(
                qT_all[:, :, g * P : (g + 1) * P], t_ps[:, 0:NH, :]
            )
            nc.vector.tensor_copy(kT[:, g * P : (g + 1) * P], t_ps[:, NH, :])

        # 2-deep software pipeline over chunks
        ropes = [None] * NG

        def emit_phase_a(g):
            if g >= 2:
                gg = g - 2
                sc = nc.named_scope(f"rope_{gg}"); sc.__enter__()
                ropes[gg] = rope_stage(gg, pend[gg][1])
                sc.__exit__(None, None, None)
            if g < NG:
                if g + 3 < NG:
                    emit_xdma(g + 3)
                if g >= 3:
                    load_wo()
                    load_wo()
                sc = nc.named_scope(f"tp_{g}"); sc.__enter__()
                xT = transposes(g)
                sc.__exit__(None, None, None)
                pend[g] = [xT, None]
            if g >= 1 and g - 1 < NG:
                gg = g - 1
                sc = nc.named_scope(f"proj_{gg}"); sc.__enter__()
                qkv_sb = proj(gg, pend[gg][0])
                sc.__exit__(None, None, None)
                pend[gg][1] = qkv_sb
            if g >= 2:
                gg = g - 2
                sc = nc.named_scope(f"ropeT_{gg}"); sc.__enter__()
                rope_transpose(gg, *ropes[gg])
                sc.__exit__(None, None, None)
                pend[gg] = None

        def scores_step(t, hp, expst):
            """scoresT + exp for head pair hp of macro tile t.

            expst free layout: [ik, 2 heads, TQ].  Causal masking is done
            post-exp by zero-filling the two diagonal chunks (affine_select
            on gpsimd); the second diagonal chunk's lower-q half (entirely
            masked) never gets a matmul."""
            qT_hp = qT_all[:, 2 * hp : 2 * hp + 2, t * TQ : (t + 1) * TQ]
            for pi in range(t + 1):
                for half in range(2):
                    ik = 2 * pi + half
                    s_ps = ps_mega.tile([P, 2, TQ], F32, tag="mega", name="s")
                    if pi == t and half == 1:
                        # keys [t*TQ+128, (t+1)*TQ): only q[128:256] can see them
                        nc.tensor.matmul(
                            s_ps[:, :, P:TQ],
                            kT[:, ik * P : (ik + 1) * P],
                            qT_hp[:, :, P:TQ],
                            start=True, stop=True,
                        )
                    else:
                        nc.tensor.matmul(
                            s_ps,
                            kT[:, ik * P : (ik + 1) * P],
                            qT_hp,
                            start=True, stop=True,
                        )
                    nc.scalar.activation(
                        out=expst[:, ik], in_=s_ps, func=AF.Exp, scale=SCALE
                    )
            # zero the causally-masked parts of the two diagonal chunks
            nc.gpsimd.affine_select(
                out=expst[:, 2 * t], in_=expst[:, 2 * t],
                compare_op=AL.is_ge, fill=0.0,
                base=0, channel_multiplier=-1, pattern=[[0, 2], [1, TQ]],
            )
            nc.gpsimd.affine_select(
                out=expst[:, 2 * t + 1], in_=expst[:, 2 * t + 1],
                compare_op=AL.is_ge, fill=0.0,
                base=-P, channel_multiplier=-1, pattern=[[0, 2], [1, TQ]],
            )

        def dnpv_step(t, hp, expst, uT_t):
            """denominator + PV matmuls, rec = exp(-ln(den)) on ACT, then
            normalize into uT_t (DVE)."""
            nsk = 2 * (t + 1)
            den_ps = ps_mega.tile([P, 2, TQ], F32, tag="mega", name="den")
            for ik in range(nsk):
                nc.tensor.matmul(
                    den_ps, ones, expst[:, ik],
                    start=(ik == 0), stop=(ik == nsk - 1),
                )
            lntmp = pb.tile([P, 2, TQ], F32, tag="lntmp", bufs=2)
            nc.scalar.activation(out=lntmp, in_=den_ps, func=AF.Ln)
            rec = pb.tile([P, 2, TQ], F32, tag="rec", bufs=2)
            nc.scalar.activation(out=rec, in_=lntmp, func=AF.Exp, scale=-1.0)
            u_ps = ps_mega.tile([P, 2, TQ], F32, tag="mega", name="u")
            for ik in range(nsk):
                nc.tensor.matmul(
                    u_ps, vv[:, ik, :], expst[:, ik],
                    start=(ik == 0), stop=(ik == nsk - 1),
                )
            nc.vector.tensor_tensor(
                out=uT_t[:, 2 * hp : 2 * hp + 2, :], in0=u_ps, in1=rec,
                op=AL.mult,
            )

        y_eng = [0]

        def wo_stage(t, uT_t):
            for sub in range(2):
                g = 2 * t + sub
                for n in range(HID // 512):
                    y_ps = ps_mega.tile([P, 512], F32, tag="mega", name="y")
                    for h in range(NH):
                        nc.tensor.matmul(
                            y_ps,
                            uT_t[:, h, sub * P : (sub + 1) * P],
                            wo_sb[:, h, n * 512 : (n + 1) * 512],
                            start=(h == 0), stop=(h == NH - 1),
                        )
                    y_sb = pb.tile([P, 512], BF16, tag="ysb", bufs=3)
                    e = y_eng[0] % 2
                    y_eng[0] += 1
                    if e == 0:
                        nc.vector.tensor_copy(y_sb, y_ps)
                    else:
                        nc.scalar.activation(out=y_sb, in_=y_ps, func=AF.Copy)
                    nc.gpsimd.dma_start(
                        out=out_d[g * P : (g + 1) * P, n * 512 : (n + 1) * 512],
                        in_=y_sb,
                    )

        # attention steps (t, hp), pipelined one deep; wo(t) fires after
        # dnpv of (t, 1)
        steps = [(t, hp) for t in range(NT) for hp in range(NHP)]
        uts = {}
        att_i = [0]

        def emit_attention_step():
            i = att_i[0]
            if i >= len(steps) + 1:
                return False
            if i < len(steps):
                t, hp = steps[i]
                if hp == 0:
                    uts[t] = pb.tile([P, NH, TQ], BF16, tag="uT", name=f"uT{t}")
                expst = pb.tile(
                    [P, NSK, 2, TQ], BF16, tag="expst", bufs=3, name=f"es{t}_{hp}"
                )
                uts[(t, hp)] = expst
                sc = nc.named_scope(f"sc_{t}_{hp}"); sc.__enter__()
                scores_step(t, hp, expst)
                sc.__exit__(None, None, None)
            if 1 <= i:
                t, hp = steps[i - 1]
                sc = nc.named_scope(f"dnpv_{t}_{hp}"); sc.__enter__()
                dnpv_step(t, hp, uts.pop((t, hp)), uts[t])
                sc.__exit__(None, None, None)
                if hp == NHP - 1:
                    sc = nc.named_scope(f"wo_{t}"); sc.__enter__()
                    wo_stage(t, uts.pop(t))
                    sc.__exit__(None, None, None)
            att_i[0] += 1
            return True

        # drive: phase-A iteration g, then any attention steps whose
        # inputs (kT/v/qT up to chunk 2t+1) are complete after ropeT_{g-2}
        for g in range(NG + 2):
            emit_phase_a(g)
            done_g = g - 2  # ropeT for this chunk just emitted
            while att_i[0] < len(steps) + 1:
                i = att_i[0]
                if i < len(steps):
                    t, _hp = steps[i]
                    if 2 * t + 1 > done_g:
                        break
                emit_attention_step()
        load_wo()
        load_wo()
        load_wo()
        load_wo()
        while emit_attention_step():
            pass

    nc.compile()
    return nc


def shard_inputs(x, cos, sin, wq, wk, wv, wo):
    """Build per-core input maps: core = b*4 + g."""
    in_maps = []
    for c in range(N_CORES):
        b, g = divmod(c, N_KV)
        in_maps.append(
            {
                "x": np.ascontiguousarray(x[b]),
                "cos": np.ascontiguousarray(cos),
                "sin": np.ascontiguousarray(sin),
                "wq": np.ascontiguousarray(wq[:, g * NH * D : (g + 1) * NH * D]),
                "wk": np.ascontiguousarray(wk[:, g * D : (g + 1) * D]),
                "wv": np.ascontiguousarray(wv[:, g * D : (g + 1) * D]),
                "wo": np.ascontiguousarray(wo[g * NH * D : (g + 1) * NH * D, :]),
            }
        )
    return in_maps


_NC_CACHE = {}


def get_nc():
    if "nc" not in _NC_CACHE:
        _NC_CACHE["nc"] = build_nc()
    return _NC_CACHE["nc"]


def kernel(x, cos, sin, wq, wk, wv, wo, _trace=False):
    from concourse.bass_utils import run_bass_kernel_spmd

    x = np.asarray(x, dtype=np.float32)
    cos = np.asarray(cos, dtype=np.float32)
    sin = np.asarray(sin, dtype=np.float32)
    wq = np.asarray(wq, dtype=np.float32)
    wk = np.asarray(wk, dtype=np.float32)
    wv = np.asarray(wv, dtype=np.float32)
    wo = np.asarray(wo, dtype=np.float32)

    nc = get_nc()
    in_maps = shard_inputs(x, cos, sin, wq, wk, wv, wo)
    res = run_bass_kernel_spmd(nc, in_maps, list(range(N_CORES)), trace=_trace)
    parts = [np.asarray(res.results[c]["out"], dtype=np.float32) for c in range(N_CORES)]
    y = np.stack(
        [sum(parts[b * N_KV + g] for g in range(N_KV)) for b in range(B)], axis=0
    )
    if _trace:
        kernel.last_result = res
    return y


# revision 12
# speedup vs baseline: 1.2319x; 1.1076x over previous
"""Trainium2 Bass kernel for GQA attention with RoPE (B=2, S=1024, HID=2048,
16 q heads / 4 kv heads, head dim 128, causal).

Sharding: 8 cores = 2 batches x 4 kv-head groups. Core c = b*4 + g handles
batch b and kv head g (query heads 4g..4g+3). Each core computes a partial
output y_part = attn_heads @ wo_shard; the host sums the 4 partials per batch.

All matmul operands are bf16 (weights cast on-chip from the fp32 DMA) so the
PE gets FWL (fast weight load) on every stationary operand and bf16 1c/row
streaming; PSUM accumulation stays fp32.  Key layout choices:

  Phase A (per 128-row chunk g, software-pipelined 2 deep):
    x chunk (fp32) --DVE cast--> x16 --PE transpose (bf16)--> xT
    qkv = xT.T @ [wq|wk|wv] (one 768-wide moving operand); RoPE on DVE in
    bf16; PE transpose q_rope/k_rope -> persistent qT[d,h,s], kT[d,s], v[s,d].
  Phase B (per (macro tile t, head pair hp), pipelined one step deep):
    per 128-key chunk ik: scoresT[sk,(2h,sq)] = kT_chunk.T @ qT_hp (512F)
    expS = exp(scale*s) (ACT) -> bf16; causal masking via gpsimd
    affine_select zero-fill on the two diagonal chunks (replaces additive
    -inf mask); denom = ones.T @ expS and U^T = v.T @ expS accumulate on PE;
    rec = exp(-ln(denom)) on ACT (DVE reciprocal is an 8-pass iterative op);
    uT = U^T * rec (DVE, bf16 out).
  Phase C: y = sum_h uT_h.T @ wo_h (PE) -> SBUF bf16 -> DRAM bf16 (host
    upcasts and sums partials in fp32).
"""

import sys

import numpy as np

for _p in ("/opt/trn_rl_repo", "/root/.axon_site/_ro/trn_rl_repo"):
    if _p not in sys.path:
        sys.path.append(_p)

from contextlib import ExitStack

import concourse.bass as bass
import concourse.mybir as mybir
from concourse import bacc
from concourse.masks import make_identity
from concourse.tile import TileContext

P = 128           # partitions / head dim / seq chunk
S = 1024          # sequence length
HID = 2048        # model dim
NH = 4            # query heads per core
D = 128           # head dim
TQ = 256          # query macro-tile
NT = S // TQ      # 4 macro tiles
KC = HID // P     # 16 contraction chunks
NSK = S // P      # 8 key chunks
NG = S // P       # 8 row chunks
NHP = NH // 2     # head pairs per core
F32 = mybir.dt.float32
BF16 = mybir.dt.bfloat16
SCALE = 1.0 / float(np.sqrt(D))
AL = mybir.AluOpType
AF = mybir.ActivationFunctionType

N_CORES = 8
B = 2
N_KV = 4

QKV = NH * D + 2 * D   # 768: q(512) | k(128) | v(128)


class _Bacc(bacc.Bacc):
    """Bacc with activation-table selection pinned to the one set that
    contains every function this kernel uses (exp, ln, copy, identity).

    The default per-instruction chooser picks the first set containing
    each function, which alternates between `exp_and_others` and
    `natural_log`, paying a ~1.3us ACT_TABLE_LOAD on every switch.  Keep
    the set list (and thus `act_func_set_id` indices) intact but empty
    every other set so the chooser can only pick the combined one.
    """

    def insert_act_table_loads(self):
        has_activation = any(
            isinstance(i, mybir.InstActivation)
            for b in self.main_func.blocks
            for i in b.instructions
        )
        if not has_activation:
            return
        from concourse.hw_specs import get_activation_tables

        tables = [
            (name, funcs if name == "natural_log_exp_and_others" else set())
            for name, funcs in get_activation_tables(self.m.arch).items()
        ]
        bacc._bass_rust.insert_act_table_loads(self, tables)


def build_nc():
    nc = _Bacc("TRN2", target_bir_lowering=False, debug=False)
    x_d = nc.declare_dram_parameter("x", [S, HID], F32, isOutput=False)
    cos_d = nc.declare_dram_parameter("cos", [S, D], F32, isOutput=False)
    sin_d = nc.declare_dram_parameter("sin", [S, D], F32, isOutput=False)
    wq_d = nc.declare_dram_parameter("wq", [HID, NH * D], F32, isOutput=False)
    wk_d = nc.declare_dram_parameter("wk", [HID, D], F32, isOutput=False)
    wv_d = nc.declare_dram_parameter("wv", [HID, D], F32, isOutput=False)
    wo_d = nc.declare_dram_parameter("wo", [NH * D, HID], F32, isOutput=False)
    out_d = nc.declare_dram_parameter("out", [S, HID], BF16, isOutput=True)

    with TileContext(nc) as tc, ExitStack() as ctx:
        consts = ctx.enter_context(tc.tile_pool(name="consts", bufs=1))
        wpool = ctx.enter_context(tc.tile_pool(name="wpool", bufs=1))
        stage = ctx.enter_context(tc.tile_pool(name="stage", bufs=2))
        persist = ctx.enter_context(tc.tile_pool(name="persist", bufs=1))

        # ---- constants ----
        ident = consts.tile([P, P], BF16, tag="ident")
        make_identity(nc, ident)
        ones = consts.tile([P, P], BF16, tag="ones")
        nc.vector.memset(ones, 1.0)

        # ---- weights: fp32 staging DMA -> bf16 SBUF cast (DVE) ----
        wqkv_sb = wpool.tile([P, KC, QKV], BF16, tag="wqkv")
        wo_sb = wpool.tile([P, NH, HID], BF16, tag="wo")
        cos_sb = wpool.tile([P, NG, D], BF16, tag="cos")
        sin_sb = wpool.tile([P, NG, D], BF16, tag="sin")

        wq_r = wq_d[:].rearrange("(c p) n -> p c n", p=P)
        wk_r = wk_d[:].rearrange("(c p) n -> p c n", p=P)
        wv_r = wv_d[:].rearrange("(c p) n -> p c n", p=P)
        wo_r = wo_d[:].rearrange("(h p) n -> p h n", p=P)

        def load_wq_round(r):
            st = stage.tile([P, 4, NH * D], F32, tag="wstage", name=f"wqs{r}")
            nc.sync.dma_start(out=st, in_=wq_r[:, 4 * r : 4 * (r + 1), :])
            nc.vector.tensor_copy(
                wqkv_sb[:, 4 * r : 4 * (r + 1), 0 : NH * D], st
            )

        def load_wkv(which):
            src, off = (wk_r, NH * D) if which == "k" else (wv_r, NH * D + D)
            st = stage.tile([P, KC, D], F32, tag="wstage", name=f"w{which}s")
            nc.sync.dma_start(out=st, in_=src)
            nc.vector.tensor_copy(wqkv_sb[:, :, off : off + D], st)

        def load_cs(which):
            src, dst = (cos_d, cos_sb) if which == "c" else (sin_d, sin_sb)
            st = stage.tile([P, NG, D], F32, tag="csstage", name=f"{which}s")
            nc.sync.dma_start(out=st, in_=src[:].rearrange("(c p) d -> p c d", p=P))
            nc.vector.tensor_copy(dst, st)

        wo_next = [0]

        def load_wo():
            h = wo_next[0]
            if h < NH:
                st = stage.tile([P, HID], F32, tag="wostage", name=f"wos{h}")
                nc.sync.dma_start(out=st, in_=wo_r[:, h, :])
                nc.vector.tensor_copy(wo_sb[:, h, :], st)
                wo_next[0] += 1

        # persistent transposed activations (all bf16)
        qT_all = persist.tile([P, NH, S], BF16, tag="qT")   # [d, h, sq]
        kT = persist.tile([P, S], BF16, tag="kT")           # [d, sk]
        vv = persist.tile([P, NSK, D], BF16, tag="vv")      # v natural [sk, d]

        H2 = D // 2

        def rope(dst, src, g, tmp_tag, wk):
            """dst = src*cos + rotate_half(src)*sin, natural layout [P, D]."""
            cos_g = cos_sb[:, g, :]
            sin_g = sin_sb[:, g, :]
            tmp = wk.tile([P, D], BF16, tag=tmp_tag)
            nc.vector.scalar_tensor_tensor(
                out=tmp[:, 0:H2], in0=src[:, H2:D], scalar=-1.0,
                in1=sin_g[:, 0:H2], op0=AL.mult, op1=AL.mult,
            )
            nc.vector.tensor_tensor(
                out=tmp[:, H2:D], in0=src[:, 0:H2], in1=sin_g[:, H2:D], op=AL.mult
            )
            nc.vector.tensor_tensor(out=dst, in0=src, in1=cos_g, op=AL.mult)
            nc.vector.tensor_tensor(out=dst, in0=dst, in1=tmp, op=AL.add)

        # ================= fused pipeline =================
        pa = ctx.enter_context(tc.tile_pool(name="pa", bufs=2))
        pb = ctx.enter_context(tc.tile_pool(name="pb", bufs=2))
        ps_mega = ctx.enter_context(tc.tile_pool(name="ps_mega", bufs=6, space="PSUM"))
        ps_qkv = ctx.enter_context(tc.tile_pool(name="ps_qkv", bufs=1, space="PSUM"))

        # dummy matmuls to lift the PE HAM clock gate to 8/8 while the
        # first x/weight DMAs are still in flight
        warm_ps = ps_mega.tile([P, 512], F32, tag="mega", name="warm")
        for _ in range(40):
            nc.tensor.matmul(warm_ps[:, 0:P], ones, ones, start=True, stop=True)
        warm_drain = pa.tile([P, 4], F32, tag="warmdrain", bufs=1)
        nc.vector.tensor_copy(warm_drain, warm_ps[:, 0:4])

        x_tiles = [None] * NG
        pend = [None] * NG  # g -> [xT, qkv_sb]

        def emit_xdma(g):
            x_nat = pa.tile([P, HID], F32, tag="xnat", bufs=2)
            nc.sync.dma_start(out=x_nat, in_=x_d[g * P : (g + 1) * P, :])
            # cast immediately so the bf16 copy isn't queued behind weight
            # casts on the DVE
            x16 = pa.tile([P, HID], BF16, tag="x16", bufs=3, name=f"x16_{g}")
            nc.vector.tensor_copy(x16, x_nat)
            x_tiles[g] = x16

        # DMA order: x0, wq rounds + casts, x1, wkv, x2, cos/sin, x3.., wo
        emit_xdma(0)
        load_wq_round(0)
        load_wq_round(1)
        emit_xdma(1)
        load_wq_round(2)
        load_wq_round(3)
        load_wkv("k")
        load_wkv("v")
        emit_xdma(2)
        load_cs("c")
        load_cs("s")

        def transposes(g):
            """xT via PE transposes of the pre-cast bf16 x chunk."""
            x16 = x_tiles[g]
            xT = pa.tile([P, KC, P], BF16, tag="xT", bufs=2)
            xT_flat = xT.rearrange("p c d -> p (c d)")
            for kb in range(2):
                tp_ps = ps_mega.tile([P, 8, P], BF16, tag="mega", name="tp")
                for j in range(8):
                    k = 8 * kb + j
                    nc.tensor.transpose(
                        tp_ps[:, j, :], x16[:, k * P : (k + 1) * P], ident
                    )
                tp_flat = tp_ps.rearrange("p c d -> p (c d)")
                if kb == 0:
                    nc.vector.tensor_copy(
                        xT_flat[:, 0 : 8 * P], tp_flat
                    )
                else:
                    nc.scalar.activation(
                        out=xT_flat[:, 8 * P : 16 * P], in_=tp_flat, func=AF.Copy
                    )
            return xT

        def proj(g, xT):
            """qkv projection: q (512F) and kv (256F) accumulation groups
            (a single matmul's PSUM output must fit one 512-fp32 bank)."""
            qkv_ps = ps_qkv.tile([P, QKV], F32, tag="qkv")
            q_ps = qkv_ps[:, 0 : NH * D]
            kv_ps = qkv_ps[:, NH * D : QKV]
            for k in range(KC):
                nc.tensor.matmul(
                    q_ps, xT[:, k, :], wqkv_sb[:, k, 0 : NH * D],
                    start=(k == 0), stop=(k == KC - 1),
                )
            for k in range(KC):
                nc.tensor.matmul(
                    kv_ps, xT[:, k, :], wqkv_sb[:, k, NH * D : QKV],
                    start=(k == 0), stop=(k == KC - 1),
                )
            qkv_sb = pa.tile([P, QKV], BF16, tag="qkvsb")
            nc.scalar.activation(out=qkv_sb, in_=qkv_ps, func=AF.Copy)
            return qkv_sb

        def rope_stage(g, qkv_sb):
            """RoPE on q heads + k (DVE, bf16), v copy-out."""
            q3 = qkv_sb[:, 0 : NH * D].rearrange("p (h d) -> p h d", h=NH)
            q_rope = pa.tile([P, NH, D], BF16, tag="qrope")
            for h in range(NH):
                rope(q_rope[:, h, :], q3[:, h, :], g, "tmq", pa)
            k_rope = pa.tile([P, D], BF16, tag="krope")
            rope(k_rope, qkv_sb[:, NH * D : NH * D + D], g, "tmk", pa)
            nc.vector.tensor_copy(
                vv[:, g, :], qkv_sb[:, NH * D + D : NH * D + 2 * D]
            )
            return q_rope, k_rope

        def rope_transpose(g, q_rope, k_rope):
            """Transpose RoPE'd q/k into persistent qT_all / kT."""
            t_ps = ps_mega.tile([P, 8, P], BF16, tag="mega", name="tq")
            for h in range(NH):
                nc.tensor.transpose(t_ps[:, h, :], q_rope[:, h, :], ident)
            nc.tensor.transpose(t_ps[:, NH, :], k_rope, ident)
            nc.vector.tensor_copy(
                qT_all[:, :, g * P : (g + 1) * P], t_ps[:, 0:NH, :]
            )
            nc.vector.tensor_copy(kT[:, g * P : (g + 1) * P], t_ps[:, NH, :])

        # 2-deep software pipeline over chunks
        ropes = [None] * NG

        def emit_phase_a(g):
            if g >= 2:
                gg = g - 2
                sc = nc.named_scope(f"rope_{gg}"); sc.__enter__()
                ropes[gg] = rope_stage(gg, pend[gg][1])
                sc.__exit__(None, None, None)
            if g < NG:
                if g + 3 < NG:
                    emit_xdma(g + 3)
                if g >= 3:
                    load_wo()
                    load_wo()
                sc = nc.named_scope(f"tp_{g}"); sc.__enter__()
                xT = transposes(g)
                sc.__exit__(None, None, None)
                pend[g] = [xT, None]
            if g >= 1 and g - 1 < NG:
                gg = g - 1
                sc = nc.named_scope(f"proj_{gg}"); sc.__enter__()
                qkv_sb = proj(gg, pend[gg][0])
                sc.__exit__(None, None, None)
                pend[gg][1] = qkv_sb
            if g >= 2:
                gg = g - 2
                sc = nc.named_scope(f"ropeT_{gg}"); sc.__enter__()
                rope_transpose(gg, *ropes[gg])
                sc.__exit__(None, None, None)
                pend[gg] = None

        def scores_step(t, hp, expst):
            """scoresT + exp for head pair hp of macro tile t.

            expst free layout: [ik, 2 heads, TQ].  Causal masking is done
            post-exp by zero-filling the two diagonal chunks (affine_select
            on gpsimd); the second diagonal chunk's lower-q half (entirely
            masked) never gets a matmul."""
            qT_hp = qT_all[:, 2 * hp : 2 * hp + 2, t * TQ : (t + 1) * TQ]
            for pi in range(t + 1):
                for half in range(2):
                    ik = 2 * pi + half
                    s_ps = ps_mega.tile([P, 2, TQ], F32, tag="mega", name="s")
                    if pi == t and half == 1:
                        # keys [t*TQ+128, (t+1)*TQ): only q[128:256] can see them
                        nc.tensor.matmul(
                            s_ps[:, :, P:TQ],
                            kT[:, ik * P : (ik + 1) * P],
                            qT_hp[:, :, P:TQ],
                            start=True, stop=True,
                        )
                    else:
                        nc.tensor.matmul(
                            s_ps,
                            kT[:, ik * P : (ik + 1) * P],
                            qT_hp,
                            start=True, stop=True,
                        )
                    nc.scalar.activation(
                        out=expst[:, ik], in_=s_ps, func=AF.Exp, scale=SCALE
                    )
            # zero the causally-masked parts of the two diagonal chunks
            nc.gpsimd.affine_select(
                out=expst[:, 2 * t], in_=expst[:, 2 * t],
                compare_op=AL.is_ge, fill=0.0,
                base=0, channel_multiplier=-1, pattern=[[0, 2], [1, TQ]],
            )
            nc.gpsimd.affine_select(
                out=expst[:, 2 * t + 1], in_=expst[:, 2 * t + 1],
                compare_op=AL.is_ge, fill=0.0,
                base=-P, channel_multiplier=-1, pattern=[[0, 2], [1, TQ]],
            )

        def dnpv_step(t, hp, expst, uT_t):
            """denominator + PV matmuls; u/den division on gpsimd (keeps the
            ACT engine on a single exp-family table set, and avoids the
            8-pass iterative DVE reciprocal)."""
            nsk = 2 * (t + 1)
            den_ps = ps_mega.tile([P, 2, TQ], F32, tag="mega", name="den")
            for ik in range(nsk):
                nc.tensor.matmul(
                    den_ps, ones, expst[:, ik],
                    start=(ik == 0), stop=(ik == nsk - 1),
                )
            lntmp = pb.tile([P, 2, TQ], F32, tag="lntmp", bufs=2)
            nc.scalar.activation(out=lntmp, in_=den_ps, func=AF.Ln)
            rec = pb.tile([P, 2, TQ], F32, tag="rec", bufs=2)
            nc.scalar.activation(out=rec, in_=lntmp, func=AF.Exp, scale=-1.0)
            u_ps = ps_mega.tile([P, 2, TQ], F32, tag="mega", name="u")
            for ik in range(nsk):
                nc.tensor.matmul(
                    u_ps, vv[:, ik, :], expst[:, ik],
                    start=(ik == 0), stop=(ik == nsk - 1),
                )
            nc.vector.tensor_tensor(
                out=uT_t[:, 2 * hp : 2 * hp + 2, :], in0=u_ps, in1=rec,
                op=AL.mult,
            )

        y_eng = [0]

        def wo_stage(t, uT_t):
            for sub in range(2):
                g = 2 * t + sub
                for n in range(HID // 512):
                    y_ps = ps_mega.tile([P, 512], F32, tag="mega", name="y")
                    for h in range(NH):
                        nc.tensor.matmul(
                            y_ps,
                            uT_t[:, h, sub * P : (sub + 1) * P],
                            wo_sb[:, h, n * 512 : (n + 1) * 512],
                            start=(h == 0), stop=(h == NH - 1),
                        )
                    y_sb = pb.tile([P, 512], BF16, tag="ysb", bufs=4)
                    e = y_eng[0] % 2
                    y_eng[0] += 1
                    if e == 0:
                        nc.vector.tensor_copy(y_sb, y_ps)
                        dma_eng = nc.gpsimd
                    else:
                        nc.scalar.activation(out=y_sb, in_=y_ps, func=AF.Copy)
                        dma_eng = nc.sync
                    dma_eng.dma_start(
                        out=out_d[g * P : (g + 1) * P, n * 512 : (n + 1) * 512],
                        in_=y_sb,
                    )

        # attention steps (t, hp), pipelined one deep; wo(t) lags one more
        # slot behind dnpv of (t, 1) so its LDWEIGHTS wait on uT doesn't
        # stall the PE queue ahead of the next scores
        steps = [(t, hp) for t in range(NT) for hp in range(NHP)]
        uts = {}
        att_i = [0]

        def emit_attention_step():
            i = att_i[0]
            if i >= len(steps) + 2:
                return False
            if i < len(steps):
                t, hp = steps[i]
                if hp == 0:
                    uts[t] = pb.tile([P, NH, TQ], BF16, tag="uT", name=f"uT{t}")
                expst = pb.tile(
                    [P, NSK, 2, TQ], BF16, tag="expst", bufs=3, name=f"es{t}_{hp}"
                )
                uts[(t, hp)] = expst
                sc = nc.named_scope(f"sc_{t}_{hp}"); sc.__enter__()
                scores_step(t, hp, expst)
                sc.__exit__(None, None, None)
            if 1 <= i < len(steps) + 1:
                t, hp = steps[i - 1]
                sc = nc.named_scope(f"dnpv_{t}_{hp}"); sc.__enter__()
                dnpv_step(t, hp, uts.pop((t, hp)), uts[t])
                sc.__exit__(None, None, None)
            if 2 <= i:
                t, hp = steps[i - 2]
                if hp == NHP - 1:
                    sc = nc.named_scope(f"wo_{t}"); sc.__enter__()
                    wo_stage(t, uts.pop(t))
                    sc.__exit__(None, None, None)
            att_i[0] += 1
            return True

        # drive: phase-A iteration g, then any attention steps whose
        # inputs (kT/v/qT up to chunk 2t+1) are complete after ropeT_{g-2}
        for g in range(NG + 2):
            emit_phase_a(g)
            done_g = g - 2  # ropeT for this chunk just emitted
            while att_i[0] < len(steps) + 2:
                i = att_i[0]
                if i < len(steps):
                    t, _hp = steps[i]
                    if 2 * t + 1 > done_g:
                        break
                emit_attention_step()
        load_wo()
        load_wo()
        load_wo()
        load_wo()
        while emit_attention_step():
            pass

    nc.compile()
    return nc


def shard_inputs(x, cos, sin, wq, wk, wv, wo):
    """Build per-core input maps: core = b*4 + g."""
    in_maps = []
    for c in range(N_CORES):
        b, g = divmod(c, N_KV)
        in_maps.append(
            {
                "x": np.ascontiguousarray(x[b]),
                "cos": np.ascontiguousarray(cos),
                "sin": np.ascontiguousarray(sin),
                "wq": np.ascontiguousarray(wq[:, g * NH * D : (g + 1) * NH * D]),
                "wk": np.ascontiguousarray(wk[:, g * D : (g + 1) * D]),
                "wv": np.ascontiguousarray(wv[:, g * D : (g + 1) * D]),
                "wo": np.ascontiguousarray(wo[g * NH * D : (g + 1) * NH * D, :]),
            }
        )
    return in_maps


_NC_CACHE = {}


def get_nc():
    if "nc" not in _NC_CACHE:
        _NC_CACHE["nc"] = build_nc()
    return _NC_CACHE["nc"]


def kernel(x, cos, sin, wq, wk, wv, wo, _trace=False):
    from concourse.bass_utils import run_bass_kernel_spmd

    x = np.asarray(x, dtype=np.float32)
    cos = np.asarray(cos, dtype=np.float32)
    sin = np.asarray(sin, dtype=np.float32)
    wq = np.asarray(wq, dtype=np.float32)
    wk = np.asarray(wk, dtype=np.float32)
    wv = np.asarray(wv, dtype=np.float32)
    wo = np.asarray(wo, dtype=np.float32)

    nc = get_nc()
    in_maps = shard_inputs(x, cos, sin, wq, wk, wv, wo)
    res = run_bass_kernel_spmd(nc, in_maps, list(range(N_CORES)), trace=_trace)
    parts = [np.asarray(res.results[c]["out"], dtype=np.float32) for c in range(N_CORES)]
    y = np.stack(
        [sum(parts[b * N_KV + g] for g in range(N_KV)) for b in range(B)], axis=0
    )
    if _trace:
        kernel.last_result = res
    return y
